# revision 1
# baseline (speedup 1.0000x reference)
"""Multi-head attention (B=2, S=2048, D=1024, H=16) on 8 Trainium2 cores.

Sharding: core c handles batch b = c//4 and head group g = c%4 (4 heads each).
Each core computes its heads' attention output and a partial output
projection [S, D]; the host sums the 4 partials per batch (the "all-reduce"
after W_o done host-side).

Device-kernel math per core (everything f32):
  Q.T = (s_b * W_q[rows]) @ X_b.T        [128=2heads*64, S] per head-pair
  K.T =  W_k[rows] @ X_b.T               (s_b = 1/8 folded into W_q on host)
  V   =  X_b @ W_v[rows].T               [S, 256] token-major
  S.T tile = K_tile @ Q.T                -> exp() -> E.T (no max-subtract:
       scores are O(5) so exp is safe in f32; softmax shift-invariance
       makes this equal to the reference up to rounding)
  PV: out[q,0:65] = sum_k E[q,k] * [V*z | z][k]  (ones-column trick:
       col 64 accumulates the softmax denominator; key-mask z zeroes
       masked keys' V rows so no mask pass over the S x S matrix)
  attn = num / den ; transpose via PE ; partial = attn @ W_o[:, rows].T
Edge case valid_len == 0: host sets s_b = 0 and z = ones -> E = 1
  -> uniform attention over all keys, exactly matching the reference.
"""

import sys

if "/opt/trn_rl_repo" not in sys.path:
    sys.path.insert(0, "/opt/trn_rl_repo")

import numpy as np
from contextlib import ExitStack

import concourse.bass as bass
import concourse.tile as tile
from concourse import bacc, mybir
from concourse import bass_utils

F32 = mybir.dt.float32
BF16 = mybir.dt.bfloat16
EXP = mybir.ActivationFunctionType.Exp

B, S, D = 2, 2048, 1024
H, DK = 16, 64
HPC = 4            # heads per core
HC = HPC * DK      # head-group width = 256
N_CORES = 8
PT = 128           # partitions
NTT = S // PT      # 16 token tiles
NFC = D // PT      # 8 feature chunks
NQC = S // 512     # 4 q-chunks of 512


def _emit_io_only(tc, xt, wq, wk, wv, wo, zt, out):
    # benchmarking aid: same I/O signature, no compute
    nc = tc.nc
    with ExitStack() as ctx:
        sb = ctx.enter_context(tc.tile_pool(name="sb", bufs=1))
        for fc in range(NFC):
            t = sb.tile([PT, S], F32, name=f"xts{fc}")
            nc.sync.dma_start(t[:], xt[fc * PT:(fc + 1) * PT, :])
        w = sb.tile([PT, HC], F32, name="w")
        nc.sync.dma_start(w[:], wq[0:PT, :])
        z = sb.tile([PT, D], F32, name="z")
        nc.vector.memset(z[:], 0.0)
        for tt in range(NTT):
            nc.sync.dma_start(out[tt * PT:(tt + 1) * PT, :], z[:])


def _emit(tc, xt, wq, wk, wv, wo, zt, out, phases=(1, 2, 3, 4)):
    nc = tc.nc
    with ExitStack() as ctx:
        sb = ctx.enter_context(tc.tile_pool(name="sb", bufs=1))

        # ---- resident inputs ----
        xts = []
        for fc in range(NFC):
            t = sb.tile([PT, S], F32, name=f"xts{fc}")
            nc.sync.dma_start(t[:], xt[fc * PT:(fc + 1) * PT, :])
            xts.append(t)

        def load_w(ap, nm):
            ws = []
            for fc in range(NFC):
                t = sb.tile([PT, HC], F32, name=f"{nm}{fc}")
                nc.sync.dma_start(t[:], ap[fc * PT:(fc + 1) * PT, :])
                ws.append(t)
            return ws

        wqs, wks, wvs = load_w(wq, "wqs"), load_w(wk, "wks"), load_w(wv, "wvs")
        wos = []
        for c in range(2):
            t = sb.tile([PT, D], F32, name=f"wos{c}")
            nc.sync.dma_start(t[:], wo[c * PT:(c + 1) * PT, :])
            wos.append(t)
        ztt = sb.tile([PT, NTT], F32, name="ztt")
        nc.sync.dma_start(ztt[:], zt[:])

        # ---- resident intermediates ----
        qk_sb = {}
        for nm in ("q", "k"):
            for p in range(2):
                qk_sb[nm, p] = sb.tile([PT, S], BF16, name=f"{nm}sb{p}")
        vzs = [sb.tile([PT, NTT, DK + 1], BF16, name=f"vz{h}") for h in range(HPC)]
        attnT = [sb.tile([PT, S], F32, name=f"attnT{c}") for c in range(2)]

        # ---- phase 1a: Q.T, K.T (2 heads stacked per 128-row tile) ----
        if 1 not in phases:
            return
        with tc.tile_pool(name="ps_qk", bufs=3, space="PSUM") as pq:
            for nm, ws in (("q", wqs), ("k", wks)):
                for p in range(2):
                    for qc in range(NQC):
                        pt = pq.tile([PT, 512], F32, name="pqk", tag="pqk")
                        for fc in range(NFC):
                            nc.tensor.matmul(
                                pt[:],
                                ws[fc][:, p * PT:(p + 1) * PT],
                                xts[fc][:, qc * 512:(qc + 1) * 512],
                                start=(fc == 0), stop=(fc == NFC - 1),
                            )
                        nc.vector.tensor_copy(
                            qk_sb[nm, p][:, qc * 512:(qc + 1) * 512], pt[:]
                        )

        # ---- phase 1b: V token-major, masked by z, plus ones(z) column ----
        with tc.tile_pool(name="ps_v", bufs=3, space="PSUM") as pv:
            for tt in range(NTT):
                pvt = pv.tile([PT, HC], F32, name="pvt", tag="pvt")
                for fc in range(NFC):
                    nc.tensor.matmul(
                        pvt[:],
                        xts[fc][:, tt * PT:(tt + 1) * PT],
                        wvs[fc][:],
                        start=(fc == 0), stop=(fc == NFC - 1),
                    )
                for h in range(HPC):
                    nc.vector.tensor_scalar_mul(
                        vzs[h][:, tt, 0:DK],
                        pvt[:, h * DK:(h + 1) * DK],
                        ztt[:, tt:tt + 1],
                    )
            for h in range(HPC):
                nc.vector.tensor_copy(vzs[h][:, :, DK], ztt[:])

        # ---- phase 2: scores -> exp -> (E @ Vz).T with tiny stationary Vz ----
        # pv psum is [65, q]: rows 0:64 = attn.T numerator, row 64 = denom.
        # Division by the free-dim denominator row via partition_broadcast.
        if 2 not in phases:
            return
        with tc.tile_pool(name="ps_s", bufs=3, space="PSUM") as pss, \
             tc.tile_pool(name="ps_p", bufs=2, space="PSUM") as psp, \
             tc.tile_pool(name="etp", bufs=4) as etp, \
             tc.tile_pool(name="rpp", bufs=4) as rpp, \
             tc.tile_pool(name="bpp", bufs=4) as bpp:
            for h in range(HPC):
                po = (h % 2) * DK
                qa = qk_sb["q", h // 2][po:po + DK, :]
                ka = qk_sb["k", h // 2][po:po + DK, :]
                for qc in range(NQC):
                    pp = psp.tile([DK + 1, 512], F32, name="pp", tag="pp")
                    for kt in range(NTT):
                        st = pss.tile([PT, 512], F32, name="st", tag="st")
                        nc.tensor.matmul(
                            st[:],
                            ka[:, kt * PT:(kt + 1) * PT],
                            qa[:, qc * 512:(qc + 1) * 512],
                            start=True, stop=True,
                        )
                        et = etp.tile([PT, 512], BF16, name="et", tag="et")
                        nc.scalar.activation(et[:], st[:], EXP)
                        nc.tensor.matmul(
                            pp[:],
                            vzs[h][:, kt, :],
                            et[:],
                            start=(kt == 0), stop=(kt == NTT - 1),
                        )
                    rr = rpp.tile([1, 512], F32, name="rr", tag="rr")
                    nc.vector.reciprocal(rr[:], pp[DK:DK + 1, :])
                    rb = bpp.tile([DK, 512], F32, name="rb", tag="rb")
                    nc.gpsimd.partition_broadcast(rb[:], rr[:])
                    nc.vector.tensor_mul(
                        attnT[h // 2][po:po + DK, qc * 512:(qc + 1) * 512],
                        pp[0:DK, :],
                        rb[:],
                    )

        # ---- phase 4: partial = attn @ W_o[:, rows].T ----
        if 4 not in phases:
            return
        with tc.tile_pool(name="ps_o", bufs=3, space="PSUM") as pso, \
             tc.tile_pool(name="stg", bufs=3) as stg:
            for tt in range(NTT):
                pot = pso.tile([PT, D], F32, name="pot", tag="pot")
                for half in range(2):
                    for c in range(2):
                        nc.tensor.matmul(
                            pot[:, half * 512:(half + 1) * 512],
                            attnT[c][:, tt * PT:(tt + 1) * PT],
                            wos[c][:, half * 512:(half + 1) * 512],
                            start=(c == 0), stop=(c == 1),
                        )
                so = stg.tile([PT, D], F32, name="so", tag="so")
                nc.vector.tensor_copy(so[:], pot[:])
                nc.sync.dma_start(out[tt * PT:(tt + 1) * PT, :], so[:])


def build(io_only=False, phases=(1, 2, 3, 4), repeat=1):
    nc = bacc.Bacc(
        "TRN2",
        target_bir_lowering=False,
        debug=False,
        enable_asserts=True,
        num_devices=N_CORES,
    )
    xt = nc.dram_tensor("xt", [D, S], F32, kind="ExternalInput").ap()
    wq = nc.dram_tensor("wq", [D, HC], F32, kind="ExternalInput").ap()
    wk = nc.dram_tensor("wk", [D, HC], F32, kind="ExternalInput").ap()
    wv = nc.dram_tensor("wv", [D, HC], F32, kind="ExternalInput").ap()
    wo = nc.dram_tensor("wo", [HC, D], F32, kind="ExternalInput").ap()
    zt = nc.dram_tensor("zt", [PT, NTT], F32, kind="ExternalInput").ap()
    out = nc.dram_tensor("out", [S, D], F32, kind="ExternalOutput").ap()
    with tile.TileContext(nc) as tc:
        for _ in range(repeat):
            if io_only:
                _emit_io_only(tc, xt, wq, wk, wv, wo, zt, out)
            else:
                _emit(tc, xt, wq, wk, wv, wo, zt, out, phases=phases)
    nc.compile()
    return nc


_NC = None


def _get_nc():
    global _NC
    if _NC is None:
        _NC = build()
    return _NC


def make_in_maps(X, valid_lens, W_q, W_k, W_v, W_o):
    X = np.asarray(X, dtype=np.float32)
    W_q = np.asarray(W_q, dtype=np.float32)
    W_k = np.asarray(W_k, dtype=np.float32)
    W_v = np.asarray(W_v, dtype=np.float32)
    W_o = np.asarray(W_o, dtype=np.float32)
    vls = np.asarray(valid_lens).astype(np.int64)
    in_maps = []
    for c in range(N_CORES):
        b, g = divmod(c, 4)
        rows = slice(g * HC, (g + 1) * HC)
        vl = int(vls[b])
        s = 0.125 if vl > 0 else 0.0
        if vl > 0:
            z = (np.arange(S) < vl).astype(np.float32)
        else:
            z = np.ones(S, dtype=np.float32)
        in_maps.append({
            "xt": np.ascontiguousarray(X[b].T),
            "wq": np.ascontiguousarray(W_q[rows].T * s),
            "wk": np.ascontiguousarray(W_k[rows].T),
            "wv": np.ascontiguousarray(W_v[rows].T),
            "wo": np.ascontiguousarray(W_o.T[rows]),
            "zt": np.ascontiguousarray(z.reshape(NTT, PT).T),
        })
    return in_maps


def combine(outs):
    out = np.empty((B, S, D), dtype=np.float32)
    for b in range(B):
        out[b] = outs[4 * b] + outs[4 * b + 1] + outs[4 * b + 2] + outs[4 * b + 3]
    return out


def kernel(X, valid_lens, W_q, W_k, W_v, W_o):
    nc = _get_nc()
    in_maps = make_in_maps(X, valid_lens, W_q, W_k, W_v, W_o)
    res = bass_utils.run_bass_kernel_spmd(nc, in_maps, core_ids=list(range(N_CORES)))
    return combine([r["out"] for r in res.results])



# revision 7
# speedup vs baseline: 1.8499x; 1.8499x over previous
"""Multi-head attention (B=2, S=2048, D=1024, H=16) on 8 Trainium2 cores.

Sharding: core c handles batch b = c//4 and head group g = c%4 (4 heads each).
Each core computes its heads' attention output and a partial output
projection [S, D]; the host sums the 4 partials per batch (the "all-reduce"
after W_o done host-side).

All matmul operands are bf16 (fp32 runs 2-pass LOW_HIGH on the PE); PSUM
accumulation stays fp32.  The program is specialized on NKT =
max_b ceil(valid_len_b / 128): key tiles >= NKT are fully masked and
contribute exactly zero to both the softmax numerator and denominator
(the V/ones columns are pre-multiplied by the key mask z), so skipping
them is exact.  Compiled variants are cached per NKT.

Device-kernel math per core (PSUM fp32):
  Q.T = (s_b * W_q[rows]) @ X_b.T        [128=2heads*64, S] per head-pair
  K.T =  W_k[rows] @ X_b.T               (s_b = 1/8 folded into W_q on host)
  V   =  X_b @ W_v[rows].T               [S, 256] token-major, * z, bf16
  per head h, per key tile kt:
    S.T chunk [128, 2048] = K_kt @ Q.T   (f32 PSUM, 4 bank-sized matmuls)
    E.T = exp(S.T)                       (one 2048-wide ACTIVATE -> bf16 SBUF)
    pp[0:64] += Vz_kt.T @ E.T ; pp[64] += z_kt.T @ E.T   (ones-column trick)
  attn.T = pp[0:64] * broadcast(approx_recip(pp[64]))    -> bf16
  partial = attn @ W_o[:, rows].T        [S, D] fp32 out
Edge case valid_len == 0: host sets s_b = 0, z = ones, NKT = 16 -> E = 1
  -> uniform attention over all keys, exactly matching the reference.
"""

import sys

if "/opt/trn_rl_repo" not in sys.path:
    sys.path.insert(0, "/opt/trn_rl_repo")

import numpy as np
from contextlib import ExitStack

import concourse.bass as bass
import concourse.tile as tile
from concourse import bacc, mybir
from concourse import bass_utils

F32 = mybir.dt.float32
BF16 = mybir.dt.bfloat16
EXP = mybir.ActivationFunctionType.Exp
LOG = mybir.ActivationFunctionType.Ln

B, S, D = 2, 2048, 1024
H, DK = 16, 64
HPC = 4            # heads per core
HC = HPC * DK      # head-group width = 256
N_CORES = 8
PT = 128           # partitions
NTT = S // PT      # 16 token tiles
NFC = D // PT      # 8 feature chunks
NQC = S // 512     # 4 q-chunks of 512


def _emit(tc, xt, wqkv, wo, zt, out, nkt):
    nc = tc.nc
    SK = nkt * PT                       # active key span
    kws = [min(512, SK - c * 512) for c in range((SK + 511) // 512)]
    with ExitStack() as ctx:
        sb = ctx.enter_context(tc.tile_pool(name="sb", bufs=1))

        # ---- resident inputs (wqkv chunk before xts chunk so fc=0 work
        # can start while later chunks stream in) ----
        wts, xts = [], []
        for fc in range(NFC):
            w = sb.tile([PT, 3 * HC], BF16, name=f"wts{fc}")
            nc.sync.dma_start(w[:], wqkv[fc * PT:(fc + 1) * PT, :])
            wts.append(w)
            t = sb.tile([PT, S], BF16, name=f"xts{fc}")
            nc.sync.dma_start(t[:], xt[fc * PT:(fc + 1) * PT, :])
            xts.append(t)
        wos = []
        for c in range(2):
            t = sb.tile([PT, D], BF16, name=f"wos{c}")
            nc.sync.dma_start(t[:], wo[c * PT:(c + 1) * PT, :])
            wos.append(t)
        ztt = sb.tile([PT, NTT], F32, name="ztt")
        nc.sync.dma_start(ztt[:], zt[:])

        # ---- resident intermediates ----
        qsb = [sb.tile([PT, S], BF16, name=f"qsb{p}") for p in range(2)]
        ksb = [sb.tile([PT, SK], BF16, name=f"ksb{p}") for p in range(2)]
        vzs = [sb.tile([PT, nkt, DK + 1], BF16, name=f"vz{h}") for h in range(HPC)]
        attnT = [sb.tile([PT, S], BF16, name=f"attnT{c}") for c in range(2)]

        # ---- phase 1a: Q.T (full S) and K.T (first SK cols only) ----
        with tc.tile_pool(name="ps_qk", bufs=6, space="PSUM") as pq:
            for p in range(2):
                for off, dst, widths in (
                    (0, qsb[p], [512] * NQC),       # wq slice of wqkv
                    (HC, ksb[p], kws),              # wk slice of wqkv
                ):
                    pts = [
                        pq.tile([PT, 512], F32, name="pqk", tag="pqk")
                        for _ in widths
                    ]
                    for fc in range(NFC):
                        ws = wts[fc][:, off + p * PT:off + (p + 1) * PT]
                        c0 = 0
                        for c, w in enumerate(widths):
                            nc.tensor.matmul(
                                pts[c][:, 0:w],
                                ws,
                                xts[fc][:, c0:c0 + w],
                                start=(fc == 0), stop=(fc == NFC - 1),
                            )
                            c0 += w
                    c0 = 0
                    for c, w in enumerate(widths):
                        nc.vector.tensor_copy(dst[:, c0:c0 + w], pts[c][:, 0:w])
                        c0 += w

        # ---- phase 1b: V token-major (first nkt tiles), masked by z,
        # plus z as the ones-column for the softmax denominator ----
        with tc.tile_pool(name="ps_v", bufs=2, space="PSUM") as pv:
            for tt in range(nkt):
                pvt = pv.tile([PT, HC], F32, name="pvt", tag="pvt")
                for fc in range(NFC):
                    nc.tensor.matmul(
                        pvt[:],
                        xts[fc][:, tt * PT:(tt + 1) * PT],
                        wts[fc][:, 2 * HC:3 * HC],
                        start=(fc == 0), stop=(fc == NFC - 1),
                    )
                for h in range(HPC):
                    nc.vector.tensor_scalar_mul(
                        vzs[h][:, tt, 0:DK],
                        pvt[:, h * DK:(h + 1) * DK],
                        ztt[:, tt:tt + 1],
                    )
            for h in range(HPC):
                nc.vector.tensor_copy(vzs[h][:, :, DK], ztt[:, 0:nkt])

        # ---- phase 2: scores -> exp -> E @ Vz, q in halves of 1024 ----
        # Both heads of a pair are interleaved: their score matmuls have a
        # 64-row contraction at base partitions 0 / 64, so the PE runs them
        # concurrently (row tiling), and the exp of one head overlaps the
        # other head's matmuls, keeping the PE HAM-warm.
        # PSUM: st0+st1 [128,1024]f32 (2+2 banks) + pp0+pp1 [65,1024]f32
        # (2+2 banks) = 8. pp rows 0:64 = numerator, row 64 = denominator;
        # pp is evicted to SBUF with one copy so the divide chain
        # (recip -> broadcast -> mul) stays off the PV critical path.
        QH = 1024
        with tc.tile_pool(name="ps_s", bufs=1, space="PSUM") as pss, \
             tc.tile_pool(name="ps_p", bufs=1, space="PSUM") as psp, \
             tc.tile_pool(name="etp", bufs=2) as etp, \
             tc.tile_pool(name="upp", bufs=2) as upp, \
             tc.tile_pool(name="rpp", bufs=2) as rpp, \
             tc.tile_pool(name="bpp", bufs=2) as bpp:
            for qh in range(2):
                q0 = qh * QH
                for p in range(2):
                    qa = [qsb[p][j * DK:(j + 1) * DK, :] for j in range(2)]
                    ka = [ksb[p][j * DK:(j + 1) * DK, :] for j in range(2)]
                    pps = [
                        psp.tile([DK + 1, QH], F32, name=f"pp{j}", tag=f"pp{j}")
                        for j in range(2)
                    ]
                    for kt in range(nkt):
                        sts, ets = [], []
                        for j in range(2):
                            stj = pss.tile([PT, QH], F32, name=f"st{j}", tag=f"st{j}")
                            sts.append(stj)
                        for c in range(2):
                            for j in range(2):
                                nc.tensor.matmul(
                                    sts[j][:, c * 512:(c + 1) * 512],
                                    ka[j][:, kt * PT:(kt + 1) * PT],
                                    qa[j][:, q0 + c * 512:q0 + (c + 1) * 512],
                                    start=True, stop=True,
                                )
                        for j in range(2):
                            etj = etp.tile([PT, QH], BF16, name=f"et{j}", tag=f"et{j}")
                            nc.scalar.activation(etj[:], sts[j][:], EXP)
                            ets.append(etj)
                        for c in range(2):
                            for j in range(2):
                                nc.tensor.matmul(
                                    pps[j][:, c * 512:(c + 1) * 512],
                                    vzs[2 * p + j][:, kt, :],
                                    ets[j][:, c * 512:(c + 1) * 512],
                                    start=(kt == 0), stop=(kt == nkt - 1),
                                )
                    for j in range(2):
                        po = j * DK
                        u = upp.tile([DK + 1, QH], F32, name=f"u{j}", tag=f"u{j}")
                        nc.vector.tensor_copy(u[:], pps[j][:])
                        # 1/den as exp(-ln(den)) on ScalarE: ln and exp share
                        # the natural_log_exp_and_others table set, and the
                        # custom-DVE fast reciprocal is broken on this
                        # runtime (plain DVE reciprocal is 8 cyc/elem on a
                        # single partition lane - far too slow).
                        rl = rpp.tile([1, QH], F32, name="rl", tag=f"rl{j}")
                        nc.scalar.activation(rl[:], u[DK:DK + 1, :], LOG)
                        rr = rpp.tile([1, QH], F32, name="rr", tag=f"rr{j}")
                        nc.scalar.activation(rr[:], rl[:], EXP, scale=-1.0)
                        rb = bpp.tile([DK, QH], F32, name="rb", tag=f"rb{j}")
                        nc.gpsimd.partition_broadcast(rb[:], rr[:])
                        nc.vector.tensor_mul(
                            attnT[p][po:po + DK, q0:q0 + QH], u[0:DK, :], rb[:]
                        )

        # ---- phase 4: partial = attn @ W_o[:, rows].T ----
        with tc.tile_pool(name="ps_o", bufs=2, space="PSUM") as pso, \
             tc.tile_pool(name="stg", bufs=3) as stg:
            for tt in range(NTT):
                pot = pso.tile([PT, D], F32, name="pot", tag="pot")
                for c in range(2):
                    for half in range(2):
                        nc.tensor.matmul(
                            pot[:, half * 512:(half + 1) * 512],
                            attnT[c][:, tt * PT:(tt + 1) * PT],
                            wos[c][:, half * 512:(half + 1) * 512],
                            start=(c == 0), stop=(c == 1),
                        )
                so = stg.tile([PT, D], BF16, name="so", tag="so")
                nc.vector.tensor_copy(so[:], pot[:])
                nc.sync.dma_start(out[tt * PT:(tt + 1) * PT, :], so[:])


def build(nkt=NTT):
    nc = bacc.Bacc(
        "TRN2",
        target_bir_lowering=False,
        debug=False,
        enable_asserts=True,
        num_devices=N_CORES,
    )
    xt = nc.dram_tensor("xt", [D, S], BF16, kind="ExternalInput").ap()
    wqkv = nc.dram_tensor("wqkv", [D, 3 * HC], BF16, kind="ExternalInput").ap()
    wo = nc.dram_tensor("wo", [HC, D], BF16, kind="ExternalInput").ap()
    zt = nc.dram_tensor("zt", [PT, NTT], F32, kind="ExternalInput").ap()
    out = nc.dram_tensor("out", [S, D], BF16, kind="ExternalOutput").ap()
    with tile.TileContext(nc) as tc:
        _emit(tc, xt, wqkv, wo, zt, out, nkt)
    nc.compile()
    return nc


_NCS = {}


def _get_nc(nkt):
    if nkt not in _NCS:
        _NCS[nkt] = build(nkt)
    return _NCS[nkt]


def _nkt_for(vls):
    nkts = []
    for v in vls:
        v = int(v)
        nkts.append(NTT if v <= 0 else min(NTT, (v + PT - 1) // PT))
    return max(nkts)


def make_in_maps(X, valid_lens, W_q, W_k, W_v, W_o):
    import ml_dtypes

    bf16 = ml_dtypes.bfloat16
    X = np.asarray(X, dtype=np.float32)
    W_q = np.asarray(W_q, dtype=np.float32)
    W_k = np.asarray(W_k, dtype=np.float32)
    W_v = np.asarray(W_v, dtype=np.float32)
    W_o = np.asarray(W_o, dtype=np.float32)
    vls = np.asarray(valid_lens).astype(np.int64)
    in_maps = []
    for c in range(N_CORES):
        b, g = divmod(c, 4)
        rows = slice(g * HC, (g + 1) * HC)
        vl = int(vls[b])
        s = 0.125 if vl > 0 else 0.0
        if vl > 0:
            z = (np.arange(S) < vl).astype(np.float32)
        else:
            z = np.ones(S, dtype=np.float32)
        wqkv = np.concatenate(
            [W_q[rows].T * s, W_k[rows].T, W_v[rows].T], axis=1
        )
        in_maps.append({
            "xt": np.ascontiguousarray(X[b].T).astype(bf16),
            "wqkv": np.ascontiguousarray(wqkv).astype(bf16),
            "wo": np.ascontiguousarray(W_o.T[rows]).astype(bf16),
            "zt": np.ascontiguousarray(z.reshape(NTT, PT).T),
        })
    return in_maps


def combine(outs):
    out = np.empty((B, S, D), dtype=np.float32)
    for b in range(B):
        out[b] = (
            np.asarray(outs[4 * b], np.float32)
            + np.asarray(outs[4 * b + 1], np.float32)
            + np.asarray(outs[4 * b + 2], np.float32)
            + np.asarray(outs[4 * b + 3], np.float32)
        )
    return out


def kernel(X, valid_lens, W_q, W_k, W_v, W_o):
    vls = np.asarray(valid_lens).astype(np.int64)
    nc = _get_nc(_nkt_for(vls))
    in_maps = make_in_maps(X, valid_lens, W_q, W_k, W_v, W_o)
    res = bass_utils.run_bass_kernel_spmd(nc, in_maps, core_ids=list(range(N_CORES)))
    return combine([r["out"] for r in res.results])


# revision 9
# speedup vs baseline: 1.9798x; 1.0702x over previous
"""Multi-head attention (B=2, S=2048, D=1024, H=16) on 8 Trainium2 cores.

Sharding: core c handles batch b = c//4 and head group g = c%4 (4 heads each).
Each core computes its heads' attention output and a partial output
projection [S, D]; the host sums the 4 partials per batch (the "all-reduce"
after W_o done host-side).

All matmul operands are bf16 (fp32 runs 2-pass LOW_HIGH on the PE); PSUM
accumulation stays fp32.  The program is specialized on NKT =
max_b ceil(valid_len_b / 128): key tiles >= NKT are fully masked and
contribute exactly zero to both the softmax numerator and denominator
(the V/ones columns are pre-multiplied by the key mask z), so skipping
them is exact.  Compiled variants are cached per NKT.

Device-kernel math per core (PSUM fp32):
  Q.T = (s_b * W_q[rows]) @ X_b.T        [128=2heads*64, S] per head-pair
  K.T =  W_k[rows] @ X_b.T               (s_b = 1/8 folded into W_q on host)
  V   =  X_b @ W_v[rows].T               [S, 256] token-major, * z, bf16
  per head h, per key tile kt:
    S.T chunk [128, 2048] = K_kt @ Q.T   (f32 PSUM, 4 bank-sized matmuls)
    E.T = exp(S.T)                       (one 2048-wide ACTIVATE -> bf16 SBUF)
    pp[0:64] += Vz_kt.T @ E.T ; pp[64] += z_kt.T @ E.T   (ones-column trick)
  attn.T = pp[0:64] * broadcast(approx_recip(pp[64]))    -> bf16
  partial = attn @ W_o[:, rows].T        [S, D] fp32 out
Edge case valid_len == 0: host sets s_b = 0, z = ones, NKT = 16 -> E = 1
  -> uniform attention over all keys, exactly matching the reference.
"""

import sys

if "/opt/trn_rl_repo" not in sys.path:
    sys.path.insert(0, "/opt/trn_rl_repo")

import numpy as np
from contextlib import ExitStack

import concourse.bass as bass
import concourse.tile as tile
from concourse import bacc, mybir
from concourse import bass_utils

F32 = mybir.dt.float32
BF16 = mybir.dt.bfloat16
EXP = mybir.ActivationFunctionType.Exp
LOG = mybir.ActivationFunctionType.Ln

B, S, D = 2, 2048, 1024
H, DK = 16, 64
HPC = 4            # heads per core
HC = HPC * DK      # head-group width = 256
N_CORES = 8
PT = 128           # partitions
NTT = S // PT      # 16 token tiles
NFC = D // PT      # 8 feature chunks
NQC = S // 512     # 4 q-chunks of 512


def _emit(tc, xt, wqkv, wo, zt, out, nkt):
    nc = tc.nc
    SK = nkt * PT                       # active key span
    kws = [min(512, SK - c * 512) for c in range((SK + 511) // 512)]
    with ExitStack() as ctx:
        sb = ctx.enter_context(tc.tile_pool(name="sb", bufs=1))

        # ---- resident inputs (wqkv chunk before xts chunk so fc=0 work
        # can start while later chunks stream in) ----
        wts, xts = [], []
        for fc in range(NFC):
            w = sb.tile([PT, 3 * HC], BF16, name=f"wts{fc}")
            nc.sync.dma_start(w[:], wqkv[fc * PT:(fc + 1) * PT, :])
            wts.append(w)
            t = sb.tile([PT, S], BF16, name=f"xts{fc}")
            nc.sync.dma_start(t[:], xt[fc * PT:(fc + 1) * PT, :])
            xts.append(t)
        wos = []
        for c in range(2):
            t = sb.tile([PT, D], BF16, name=f"wos{c}")
            nc.sync.dma_start(t[:], wo[c * PT:(c + 1) * PT, :])
            wos.append(t)
        ztt = sb.tile([PT, NTT], F32, name="ztt")
        nc.sync.dma_start(ztt[:], zt[:])

        # ---- resident intermediates ----
        qsb = [sb.tile([PT, S], BF16, name=f"qsb{p}") for p in range(2)]
        ksb = [sb.tile([PT, SK], BF16, name=f"ksb{p}") for p in range(2)]
        vzs = [sb.tile([PT, nkt, DK + 1], BF16, name=f"vz{h}") for h in range(HPC)]
        attnT = [sb.tile([PT, S], BF16, name=f"attnT{c}") for c in range(2)]

        # ---- phase 1a: Q.T (full S) and K.T (first SK cols only) ----
        with tc.tile_pool(name="ps_qk", bufs=6, space="PSUM") as pq:
            for p in range(2):
                for off, dst, widths in (
                    (0, qsb[p], [512] * NQC),       # wq slice of wqkv
                    (HC, ksb[p], kws),              # wk slice of wqkv
                ):
                    pts = [
                        pq.tile([PT, 512], F32, name="pqk", tag="pqk")
                        for _ in widths
                    ]
                    for fc in range(NFC):
                        ws = wts[fc][:, off + p * PT:off + (p + 1) * PT]
                        c0 = 0
                        for c, w in enumerate(widths):
                            nc.tensor.matmul(
                                pts[c][:, 0:w],
                                ws,
                                xts[fc][:, c0:c0 + w],
                                start=(fc == 0), stop=(fc == NFC - 1),
                            )
                            c0 += w
                    c0 = 0
                    for c, w in enumerate(widths):
                        nc.vector.tensor_copy(dst[:, c0:c0 + w], pts[c][:, 0:w])
                        c0 += w

        # ---- phase 1b: V token-major (first nkt tiles), masked by z,
        # plus z as the ones-column for the softmax denominator ----
        with tc.tile_pool(name="ps_v", bufs=2, space="PSUM") as pv:
            for tt in range(nkt):
                pvt = pv.tile([PT, HC], F32, name="pvt", tag="pvt")
                for fc in range(NFC):
                    nc.tensor.matmul(
                        pvt[:],
                        xts[fc][:, tt * PT:(tt + 1) * PT],
                        wts[fc][:, 2 * HC:3 * HC],
                        start=(fc == 0), stop=(fc == NFC - 1),
                    )
                for h in range(HPC):
                    nc.vector.tensor_scalar_mul(
                        vzs[h][:, tt, 0:DK],
                        pvt[:, h * DK:(h + 1) * DK],
                        ztt[:, tt:tt + 1],
                    )
            for h in range(HPC):
                nc.vector.tensor_copy(vzs[h][:, :, DK], ztt[:, 0:nkt])

        # ---- phase 2: scores -> exp -> E @ Vz, q in halves of 1024 ----
        # Both heads of a pair are interleaved: their score matmuls have a
        # 64-row contraction at base partitions 0 / 64, so the PE runs them
        # concurrently (row tiling), and the exp of one head overlaps the
        # other head's matmuls, keeping the PE HAM-warm.
        # PSUM: st0+st1 [128,1024]f32 (2+2 banks) + pp0+pp1 [65,1024]f32
        # (2+2 banks) = 8. pp rows 0:64 = numerator, row 64 = denominator;
        # pp is evicted to SBUF with one copy so the divide chain
        # (recip -> broadcast -> mul) stays off the PV critical path.
        QH = 1024
        with tc.tile_pool(name="ps_s", bufs=1, space="PSUM") as pss, \
             tc.tile_pool(name="ps_p", bufs=1, space="PSUM") as psp, \
             tc.tile_pool(name="etp", bufs=3) as etp, \
             tc.tile_pool(name="upp", bufs=2) as upp, \
             tc.tile_pool(name="rpp", bufs=2) as rpp, \
             tc.tile_pool(name="bpp", bufs=2) as bpp:
            for qh in range(2):
                q0 = qh * QH
                for p in range(2):
                    qa = [qsb[p][j * DK:(j + 1) * DK, :] for j in range(2)]
                    ka = [ksb[p][j * DK:(j + 1) * DK, :] for j in range(2)]
                    pps = [
                        psp.tile([DK + 1, QH], F32, name=f"pp{j}", tag=f"pp{j}")
                        for j in range(2)
                    ]
                    for kt in range(nkt):
                        sts, ets = [], []
                        for j in range(2):
                            stj = pss.tile([PT, QH], F32, name=f"st{j}", tag=f"st{j}")
                            sts.append(stj)
                        for c in range(2):
                            for j in range(2):
                                nc.tensor.matmul(
                                    sts[j][:, c * 512:(c + 1) * 512],
                                    ka[j][:, kt * PT:(kt + 1) * PT],
                                    qa[j][:, q0 + c * 512:q0 + (c + 1) * 512],
                                    start=True, stop=True,
                                )
                        for j in range(2):
                            etj = etp.tile([PT, QH], BF16, name=f"et{j}", tag=f"et{j}")
                            nc.scalar.activation(etj[:], sts[j][:], EXP)
                            ets.append(etj)
                        for c in range(2):
                            for j in range(2):
                                nc.tensor.matmul(
                                    pps[j][:, c * 512:(c + 1) * 512],
                                    vzs[2 * p + j][:, kt, :],
                                    ets[j][:, c * 512:(c + 1) * 512],
                                    start=(kt == 0), stop=(kt == nkt - 1),
                                )
                    for j in range(2):
                        po = j * DK
                        u = upp.tile([DK + 1, QH], F32, name=f"u{j}", tag=f"u{j}")
                        nc.vector.tensor_copy(u[:], pps[j][:])
                        # 1/den: DVE reciprocal is 8 cyc/elem per partition
                        # lane, so a [1, QH] recip is 8.5us of one lane. DMA
                        # the row into a [128, QH/128] layout, recip there
                        # (128 lanes -> ~70ns), DMA back, then broadcast.
                        # (ScalarE ln/exp would thrash the activation table
                        # set against the score exps; custom-DVE approx
                        # reciprocal is broken on this runtime.)
                        dvert = rpp.tile([PT, QH // PT], F32, name="dv", tag=f"dv{j}")
                        nc.sync.dma_start(dvert[:], u[DK:DK + 1, :])
                        nc.vector.reciprocal(dvert[:], dvert[:])
                        rr = rpp.tile([1, QH], F32, name="rr", tag=f"rr{j}")
                        nc.sync.dma_start(rr[:], dvert[:])
                        rb = bpp.tile([DK, QH], F32, name="rb", tag=f"rb{j}")
                        nc.gpsimd.partition_broadcast(rb[:], rr[:])
                        nc.vector.tensor_mul(
                            attnT[p][po:po + DK, q0:q0 + QH], u[0:DK, :], rb[:]
                        )

        # ---- phase 4: partial = attn @ W_o[:, rows].T ----
        with tc.tile_pool(name="ps_o", bufs=2, space="PSUM") as pso, \
             tc.tile_pool(name="stg", bufs=3) as stg:
            for tt in range(NTT):
                pot = pso.tile([PT, D], F32, name="pot", tag="pot")
                for c in range(2):
                    for half in range(2):
                        nc.tensor.matmul(
                            pot[:, half * 512:(half + 1) * 512],
                            attnT[c][:, tt * PT:(tt + 1) * PT],
                            wos[c][:, half * 512:(half + 1) * 512],
                            start=(c == 0), stop=(c == 1),
                        )
                so = stg.tile([PT, D], BF16, name="so", tag="so")
                nc.vector.tensor_copy(so[:], pot[:])
                nc.sync.dma_start(out[tt * PT:(tt + 1) * PT, :], so[:])


def build(nkt=NTT):
    nc = bacc.Bacc(
        "TRN2",
        target_bir_lowering=False,
        debug=False,
        enable_asserts=True,
        num_devices=N_CORES,
    )
    xt = nc.dram_tensor("xt", [D, S], BF16, kind="ExternalInput").ap()
    wqkv = nc.dram_tensor("wqkv", [D, 3 * HC], BF16, kind="ExternalInput").ap()
    wo = nc.dram_tensor("wo", [HC, D], BF16, kind="ExternalInput").ap()
    zt = nc.dram_tensor("zt", [PT, NTT], F32, kind="ExternalInput").ap()
    out = nc.dram_tensor("out", [S, D], BF16, kind="ExternalOutput").ap()
    with tile.TileContext(nc) as tc:
        _emit(tc, xt, wqkv, wo, zt, out, nkt)
    nc.compile()
    return nc


_NCS = {}


def _get_nc(nkt):
    if nkt not in _NCS:
        _NCS[nkt] = build(nkt)
    return _NCS[nkt]


def _nkt_for(vls):
    nkts = []
    for v in vls:
        v = int(v)
        nkts.append(NTT if v <= 0 else min(NTT, (v + PT - 1) // PT))
    return max(nkts)


def make_in_maps(X, valid_lens, W_q, W_k, W_v, W_o):
    import ml_dtypes

    bf16 = ml_dtypes.bfloat16
    X = np.asarray(X, dtype=np.float32)
    W_q = np.asarray(W_q, dtype=np.float32)
    W_k = np.asarray(W_k, dtype=np.float32)
    W_v = np.asarray(W_v, dtype=np.float32)
    W_o = np.asarray(W_o, dtype=np.float32)
    vls = np.asarray(valid_lens).astype(np.int64)
    in_maps = []
    for c in range(N_CORES):
        b, g = divmod(c, 4)
        rows = slice(g * HC, (g + 1) * HC)
        vl = int(vls[b])
        s = 0.125 if vl > 0 else 0.0
        if vl > 0:
            z = (np.arange(S) < vl).astype(np.float32)
        else:
            z = np.ones(S, dtype=np.float32)
        wqkv = np.concatenate(
            [W_q[rows].T * s, W_k[rows].T, W_v[rows].T], axis=1
        )
        in_maps.append({
            "xt": np.ascontiguousarray(X[b].T).astype(bf16),
            "wqkv": np.ascontiguousarray(wqkv).astype(bf16),
            "wo": np.ascontiguousarray(W_o.T[rows]).astype(bf16),
            "zt": np.ascontiguousarray(z.reshape(NTT, PT).T),
        })
    return in_maps


def combine(outs):
    out = np.empty((B, S, D), dtype=np.float32)
    for b in range(B):
        out[b] = (
            np.asarray(outs[4 * b], np.float32)
            + np.asarray(outs[4 * b + 1], np.float32)
            + np.asarray(outs[4 * b + 2], np.float32)
            + np.asarray(outs[4 * b + 3], np.float32)
        )
    return out


def kernel(X, valid_lens, W_q, W_k, W_v, W_o):
    vls = np.asarray(valid_lens).astype(np.int64)
    nc = _get_nc(_nkt_for(vls))
    in_maps = make_in_maps(X, valid_lens, W_q, W_k, W_v, W_o)
    res = bass_utils.run_bass_kernel_spmd(nc, in_maps, core_ids=list(range(N_CORES)))
    return combine([r["out"] for r in res.results])


# revision 10
# speedup vs baseline: 1.9852x; 1.0027x over previous
"""Multi-head attention (B=2, S=2048, D=1024, H=16) on 8 Trainium2 cores.

Sharding: core c handles batch b = c//4 and head group g = c%4 (4 heads each).
Each core computes its heads' attention output and a partial output
projection [S, D]; the host sums the 4 partials per batch (the "all-reduce"
after W_o done host-side).

All matmul operands are bf16 (fp32 runs 2-pass LOW_HIGH on the PE); PSUM
accumulation stays fp32.  The program is specialized on NKT =
max_b ceil(valid_len_b / 128): key tiles >= NKT are fully masked and
contribute exactly zero to both the softmax numerator and denominator
(the V/ones columns are pre-multiplied by the key mask z), so skipping
them is exact.  Compiled variants are cached per NKT.

Device-kernel math per core (PSUM fp32):
  Q.T = (s_b * W_q[rows]) @ X_b.T        [128=2heads*64, S] per head-pair
  K.T =  W_k[rows] @ X_b.T               (s_b = 1/8 folded into W_q on host)
  V   =  X_b @ W_v[rows].T               [S, 256] token-major, * z, bf16
  per head h, per key tile kt:
    S.T chunk [128, 2048] = K_kt @ Q.T   (f32 PSUM, 4 bank-sized matmuls)
    E.T = exp(S.T)                       (one 2048-wide ACTIVATE -> bf16 SBUF)
    pp[0:64] += Vz_kt.T @ E.T ; pp[64] += z_kt.T @ E.T   (ones-column trick)
  attn.T = pp[0:64] * broadcast(approx_recip(pp[64]))    -> bf16
  partial = attn @ W_o[:, rows].T        [S, D] fp32 out
Edge case valid_len == 0: host sets s_b = 0, z = ones, NKT = 16 -> E = 1
  -> uniform attention over all keys, exactly matching the reference.
"""

import sys

if "/opt/trn_rl_repo" not in sys.path:
    sys.path.insert(0, "/opt/trn_rl_repo")

import numpy as np
from contextlib import ExitStack

import concourse.bass as bass
import concourse.tile as tile
from concourse import bacc, mybir
from concourse import bass_utils

F32 = mybir.dt.float32
BF16 = mybir.dt.bfloat16
EXP = mybir.ActivationFunctionType.Exp
LOG = mybir.ActivationFunctionType.Ln

B, S, D = 2, 2048, 1024
H, DK = 16, 64
HPC = 4            # heads per core
HC = HPC * DK      # head-group width = 256
N_CORES = 8
PT = 128           # partitions
NTT = S // PT      # 16 token tiles
NFC = D // PT      # 8 feature chunks
NQC = S // 512     # 4 q-chunks of 512


def _emit(tc, xt, wqkv, wo, zt, out, nkt):
    nc = tc.nc
    SK = nkt * PT                       # active key span
    kws = [min(512, SK - c * 512) for c in range((SK + 511) // 512)]
    with ExitStack() as ctx:
        sb = ctx.enter_context(tc.tile_pool(name="sb", bufs=1))

        # ---- resident inputs (wqkv chunk before xts chunk so fc=0 work
        # can start while later chunks stream in) ----
        wts, xts = [], []
        for fc in range(NFC):
            w = sb.tile([PT, 3 * HC], BF16, name=f"wts{fc}")
            nc.sync.dma_start(w[:], wqkv[fc * PT:(fc + 1) * PT, :])
            wts.append(w)
            t = sb.tile([PT, S], BF16, name=f"xts{fc}")
            nc.sync.dma_start(t[:], xt[fc * PT:(fc + 1) * PT, :])
            xts.append(t)
        wos = []
        for c in range(2):
            t = sb.tile([PT, D], BF16, name=f"wos{c}")
            nc.sync.dma_start(t[:], wo[c * PT:(c + 1) * PT, :])
            wos.append(t)
        ztt = sb.tile([PT, NTT], F32, name="ztt")
        nc.sync.dma_start(ztt[:], zt[:])

        # ---- resident intermediates ----
        qsb = [sb.tile([PT, S], BF16, name=f"qsb{p}") for p in range(2)]
        ksb = [sb.tile([PT, SK], BF16, name=f"ksb{p}") for p in range(2)]
        vzs = [sb.tile([PT, nkt, DK + 1], BF16, name=f"vz{h}") for h in range(HPC)]
        attnT = [sb.tile([PT, S], BF16, name=f"attnT{c}") for c in range(2)]

        # ---- phase 1a: Q.T (full S) and K.T (first SK cols only) ----
        with tc.tile_pool(name="ps_qk", bufs=6, space="PSUM") as pq:
            for p in range(2):
                for off, dst, widths in (
                    (0, qsb[p], [512] * NQC),       # wq slice of wqkv
                    (HC, ksb[p], kws),              # wk slice of wqkv
                ):
                    pts = [
                        pq.tile([PT, 512], F32, name="pqk", tag="pqk")
                        for _ in widths
                    ]
                    for fc in range(NFC):
                        ws = wts[fc][:, off + p * PT:off + (p + 1) * PT]
                        c0 = 0
                        for c, w in enumerate(widths):
                            nc.tensor.matmul(
                                pts[c][:, 0:w],
                                ws,
                                xts[fc][:, c0:c0 + w],
                                start=(fc == 0), stop=(fc == NFC - 1),
                            )
                            c0 += w
                    c0 = 0
                    for c, w in enumerate(widths):
                        nc.vector.tensor_copy(dst[:, c0:c0 + w], pts[c][:, 0:w])
                        c0 += w

        # ---- phase 1b: V token-major (first nkt tiles), masked by z,
        # plus z as the ones-column for the softmax denominator ----
        with tc.tile_pool(name="ps_v", bufs=2, space="PSUM") as pv:
            for tt in range(nkt):
                pvt = pv.tile([PT, HC], F32, name="pvt", tag="pvt")
                for fc in range(NFC):
                    nc.tensor.matmul(
                        pvt[:],
                        xts[fc][:, tt * PT:(tt + 1) * PT],
                        wts[fc][:, 2 * HC:3 * HC],
                        start=(fc == 0), stop=(fc == NFC - 1),
                    )
                for h in range(HPC):
                    nc.vector.tensor_scalar_mul(
                        vzs[h][:, tt, 0:DK],
                        pvt[:, h * DK:(h + 1) * DK],
                        ztt[:, tt:tt + 1],
                    )
            for h in range(HPC):
                nc.vector.tensor_copy(vzs[h][:, :, DK], ztt[:, 0:nkt])

        # ---- phase 2: scores -> exp -> E @ Vz, q in halves of 1024 ----
        # Both heads of a pair are interleaved: their score matmuls have a
        # 64-row contraction at base partitions 0 / 64, so the PE runs them
        # concurrently (row tiling), and the exp of one head overlaps the
        # other head's matmuls, keeping the PE HAM-warm.
        # PSUM: st0+st1 [128,1024]f32 (2+2 banks) + pp0+pp1 [65,1024]f32
        # (2+2 banks) = 8. pp rows 0:64 = numerator, row 64 = denominator;
        # pp is evicted to SBUF with one copy so the divide chain
        # (recip -> broadcast -> mul) stays off the PV critical path.
        QH = 1024
        with tc.tile_pool(name="ps_s", bufs=1, space="PSUM") as pss, \
             tc.tile_pool(name="ps_p", bufs=1, space="PSUM") as psp, \
             tc.tile_pool(name="etp", bufs=nkt) as etp, \
             tc.tile_pool(name="upp", bufs=2) as upp, \
             tc.tile_pool(name="rpp", bufs=2) as rpp, \
             tc.tile_pool(name="bpp", bufs=2) as bpp:

            def emit_a(g):
                # stage A: scores + exp for every kt; E tiles parked in SBUF
                qh, p = g
                q0 = qh * QH
                qa = [qsb[p][j * DK:(j + 1) * DK, :] for j in range(2)]
                ka = [ksb[p][j * DK:(j + 1) * DK, :] for j in range(2)]
                ets = []
                for kt in range(nkt):
                    sts = []
                    for j in range(2):
                        stj = pss.tile([PT, QH], F32, name=f"st{j}", tag=f"st{j}")
                        sts.append(stj)
                    for c in range(2):
                        for j in range(2):
                            nc.tensor.matmul(
                                sts[j][:, c * 512:(c + 1) * 512],
                                ka[j][:, kt * PT:(kt + 1) * PT],
                                qa[j][:, q0 + c * 512:q0 + (c + 1) * 512],
                                start=True, stop=True,
                            )
                    pair = []
                    for j in range(2):
                        etj = etp.tile([PT, QH], BF16, name=f"et{j}", tag=f"et{j}")
                        nc.scalar.activation(etj[:], sts[j][:], EXP)
                        pair.append(etj)
                    ets.append(pair)
                return ets

            def emit_b(g, ets):
                # stage B: PV accumulation + normalization; only pp banks,
                # so it overlaps the next group's stage A on the PE
                qh, p = g
                q0 = qh * QH
                pps = [
                    psp.tile([DK + 1, QH], F32, name=f"pp{j}", tag=f"pp{j}")
                    for j in range(2)
                ]
                for kt in range(nkt):
                    for c in range(2):
                        for j in range(2):
                            nc.tensor.matmul(
                                pps[j][:, c * 512:(c + 1) * 512],
                                vzs[2 * p + j][:, kt, :],
                                ets[kt][j][:, c * 512:(c + 1) * 512],
                                start=(kt == 0), stop=(kt == nkt - 1),
                            )
                for j in range(2):
                    po = j * DK
                    u = upp.tile([DK + 1, QH], F32, name=f"u{j}", tag=f"u{j}")
                    nc.vector.tensor_copy(u[:], pps[j][:])
                    # 1/den: DVE reciprocal is 8 cyc/elem per partition
                    # lane, so a [1, QH] recip is 8.5us of one lane. DMA
                    # the row into a [128, QH/128] layout, recip there
                    # (128 lanes -> ~70ns), DMA back, then broadcast.
                    # (ScalarE ln/exp would thrash the activation table
                    # set against the score exps; custom-DVE approx
                    # reciprocal is broken on this runtime.)
                    dvert = rpp.tile([PT, QH // PT], F32, name="dv", tag=f"dv{j}")
                    nc.sync.dma_start(dvert[:], u[DK:DK + 1, :])
                    nc.vector.reciprocal(dvert[:], dvert[:])
                    rr = rpp.tile([1, QH], F32, name="rr", tag=f"rr{j}")
                    nc.sync.dma_start(rr[:], dvert[:])
                    rb = bpp.tile([DK, QH], F32, name="rb", tag=f"rb{j}")
                    nc.gpsimd.partition_broadcast(rb[:], rr[:])
                    nc.vector.tensor_mul(
                        attnT[p][po:po + DK, q0:q0 + QH], u[0:DK, :], rb[:]
                    )

            groups = [(qh, p) for qh in range(2) for p in range(2)]
            prev = None
            for g in groups:
                ets = emit_a(g)
                if prev is not None:
                    emit_b(*prev)
                prev = (g, ets)
            emit_b(*prev)

        # ---- phase 4: partial = attn @ W_o[:, rows].T ----
        with tc.tile_pool(name="ps_o", bufs=2, space="PSUM") as pso, \
             tc.tile_pool(name="stg", bufs=3) as stg:
            for tt in range(NTT):
                pot = pso.tile([PT, D], F32, name="pot", tag="pot")
                for c in range(2):
                    for half in range(2):
                        nc.tensor.matmul(
                            pot[:, half * 512:(half + 1) * 512],
                            attnT[c][:, tt * PT:(tt + 1) * PT],
                            wos[c][:, half * 512:(half + 1) * 512],
                            start=(c == 0), stop=(c == 1),
                        )
                so = stg.tile([PT, D], BF16, name="so", tag="so")
                nc.vector.tensor_copy(so[:], pot[:])
                nc.sync.dma_start(out[tt * PT:(tt + 1) * PT, :], so[:])


def build(nkt=NTT):
    nc = bacc.Bacc(
        "TRN2",
        target_bir_lowering=False,
        debug=False,
        enable_asserts=True,
        num_devices=N_CORES,
    )
    xt = nc.dram_tensor("xt", [D, S], BF16, kind="ExternalInput").ap()
    wqkv = nc.dram_tensor("wqkv", [D, 3 * HC], BF16, kind="ExternalInput").ap()
    wo = nc.dram_tensor("wo", [HC, D], BF16, kind="ExternalInput").ap()
    zt = nc.dram_tensor("zt", [PT, NTT], F32, kind="ExternalInput").ap()
    out = nc.dram_tensor("out", [S, D], BF16, kind="ExternalOutput").ap()
    with tile.TileContext(nc) as tc:
        _emit(tc, xt, wqkv, wo, zt, out, nkt)
    nc.compile()
    return nc


_NCS = {}


def _get_nc(nkt):
    if nkt not in _NCS:
        _NCS[nkt] = build(nkt)
    return _NCS[nkt]


def _nkt_for(vls):
    nkts = []
    for v in vls:
        v = int(v)
        nkts.append(NTT if v <= 0 else min(NTT, (v + PT - 1) // PT))
    return max(nkts)


def make_in_maps(X, valid_lens, W_q, W_k, W_v, W_o):
    import ml_dtypes

    bf16 = ml_dtypes.bfloat16
    X = np.asarray(X, dtype=np.float32)
    W_q = np.asarray(W_q, dtype=np.float32)
    W_k = np.asarray(W_k, dtype=np.float32)
    W_v = np.asarray(W_v, dtype=np.float32)
    W_o = np.asarray(W_o, dtype=np.float32)
    vls = np.asarray(valid_lens).astype(np.int64)
    in_maps = []
    for c in range(N_CORES):
        b, g = divmod(c, 4)
        rows = slice(g * HC, (g + 1) * HC)
        vl = int(vls[b])
        s = 0.125 if vl > 0 else 0.0
        if vl > 0:
            z = (np.arange(S) < vl).astype(np.float32)
        else:
            z = np.ones(S, dtype=np.float32)
        wqkv = np.concatenate(
            [W_q[rows].T * s, W_k[rows].T, W_v[rows].T], axis=1
        )
        in_maps.append({
            "xt": np.ascontiguousarray(X[b].T).astype(bf16),
            "wqkv": np.ascontiguousarray(wqkv).astype(bf16),
            "wo": np.ascontiguousarray(W_o.T[rows]).astype(bf16),
            "zt": np.ascontiguousarray(z.reshape(NTT, PT).T),
        })
    return in_maps


def combine(outs):
    out = np.empty((B, S, D), dtype=np.float32)
    for b in range(B):
        out[b] = (
            np.asarray(outs[4 * b], np.float32)
            + np.asarray(outs[4 * b + 1], np.float32)
            + np.asarray(outs[4 * b + 2], np.float32)
            + np.asarray(outs[4 * b + 3], np.float32)
        )
    return out


def kernel(X, valid_lens, W_q, W_k, W_v, W_o):
    vls = np.asarray(valid_lens).astype(np.int64)
    nc = _get_nc(_nkt_for(vls))
    in_maps = make_in_maps(X, valid_lens, W_q, W_k, W_v, W_o)
    res = bass_utils.run_bass_kernel_spmd(nc, in_maps, core_ids=list(range(N_CORES)))
    return combine([r["out"] for r in res.results])


# revision 15
# speedup vs baseline: 2.7222x; 1.3712x over previous
"""Multi-head attention (B=2, S=2048, D=1024, H=16) on 8 Trainium2 cores.

Sharding: core c handles batch b = c//4 and head group g = c%4 (4 heads each).
Each core computes its heads' attention output and a partial output
projection [S, D]; the host sums the 4 partials per batch (the "all-reduce"
after W_o done host-side).

All matmul operands are bf16 (fp32 runs 2-pass LOW_HIGH on the PE); PSUM
accumulation stays fp32.  The program is specialized on NKT =
max_b ceil(valid_len_b / 128): key tiles >= NKT are fully masked and
contribute exactly zero to both the softmax numerator and denominator
(the V/ones columns are pre-multiplied by the key mask z), so skipping
them is exact.  Compiled variants are cached per NKT.

Device-kernel math per core (PSUM fp32):
  Q.T = (s_b * W_q[rows]) @ X_b.T        [128=2heads*64, S] per head-pair
  K.T =  W_k[rows] @ X_b.T               (s_b = 1/8 folded into W_q on host)
  V   =  X_b @ W_v[rows].T               [S, 256] token-major, * z, bf16
  per head h, per key tile kt:
    S.T chunk [128, 2048] = K_kt @ Q.T   (f32 PSUM, 4 bank-sized matmuls)
    E.T = exp(S.T)                       (one 2048-wide ACTIVATE -> bf16 SBUF)
    pp[0:64] += Vz_kt.T @ E.T ; pp[64] += z_kt.T @ E.T   (ones-column trick)
  attn.T = pp[0:64] * broadcast(approx_recip(pp[64]))    -> bf16
  partial = attn @ W_o[:, rows].T        [S, D] fp32 out
Edge case valid_len == 0: host sets s_b = 0, z = ones, NKT = 16 -> E = 1
  -> uniform attention over all keys, exactly matching the reference.
"""

import sys

if "/opt/trn_rl_repo" not in sys.path:
    sys.path.insert(0, "/opt/trn_rl_repo")

import numpy as np
from contextlib import ExitStack

import concourse.bass as bass
import concourse.tile as tile
from concourse import bacc, mybir
from concourse import bass_utils

F32 = mybir.dt.float32
BF16 = mybir.dt.bfloat16
EXP = mybir.ActivationFunctionType.Exp
LOG = mybir.ActivationFunctionType.Ln

B, S, D = 2, 2048, 1024
H, DK = 16, 64
HPC = 4            # heads per core
HC = HPC * DK      # head-group width = 256
N_CORES = 8
PT = 128           # partitions
NTT = S // PT      # 16 token tiles
NFC = D // PT      # 8 feature chunks
NQC = S // 512     # 4 q-chunks of 512


def _emit(tc, xt, wqkv, wo, zt, out, nkt):
    nc = tc.nc
    SK = nkt * PT                       # active key span
    kws = [min(512, SK - c * 512) for c in range((SK + 511) // 512)]
    with ExitStack() as ctx:
        sb = ctx.enter_context(tc.tile_pool(name="sb", bufs=1))

        # ---- resident inputs (wqkv chunk before xts chunk so fc=0 work
        # can start while later chunks stream in) ----
        wts, xts = [], []
        for fc in range(NFC):
            w = sb.tile([PT, 3 * HC], BF16, name=f"wts{fc}")
            nc.sync.dma_start(w[:], wqkv[fc * PT:(fc + 1) * PT, :])
            wts.append(w)
            t = sb.tile([PT, S], BF16, name=f"xts{fc}")
            # chunked so the first matmuls start after ~0.3 MB, not 8 MB
            for ck in range(4):
                nc.sync.dma_start(
                    t[:, ck * 512:(ck + 1) * 512],
                    xt[fc * PT:(fc + 1) * PT, ck * 512:(ck + 1) * 512],
                )
            xts.append(t)
        wos = []
        for c in range(2):
            t = sb.tile([PT, D], BF16, name=f"wos{c}")
            nc.sync.dma_start(t[:], wo[c * PT:(c + 1) * PT, :])
            wos.append(t)
        ztt = sb.tile([PT, NTT], F32, name="ztt")
        nc.sync.dma_start(ztt[:], zt[:])

        # ---- resident intermediates ----
        qsb = [sb.tile([PT, S], BF16, name=f"qsb{p}") for p in range(2)]
        ksb = [sb.tile([PT, SK], BF16, name=f"ksb{p}") for p in range(2)]
        vzs = [sb.tile([PT, nkt, DK + 1], BF16, name=f"vz{h}") for h in range(HPC)]
        attnT = [sb.tile([PT, S], BF16, name=f"attnT{c}") for c in range(2)]

        # ---- phase 1a: Q.T (full S) and K.T (first SK cols only) ----
        with tc.tile_pool(name="ps_qk", bufs=6, space="PSUM") as pq:
            for p in range(2):
                for off, dst, widths in (
                    (0, qsb[p], [512] * NQC),       # wq slice of wqkv
                    (HC, ksb[p], kws),              # wk slice of wqkv
                ):
                    pts = [
                        pq.tile([PT, 512], F32, name="pqk", tag="pqk")
                        for _ in widths
                    ]
                    for fc in range(NFC):
                        ws = wts[fc][:, off + p * PT:off + (p + 1) * PT]
                        c0 = 0
                        for c, w in enumerate(widths):
                            nc.tensor.matmul(
                                pts[c][:, 0:w],
                                ws,
                                xts[fc][:, c0:c0 + w],
                                start=(fc == 0), stop=(fc == NFC - 1),
                            )
                            c0 += w
                    c0 = 0
                    for c, w in enumerate(widths):
                        nc.vector.tensor_copy(dst[:, c0:c0 + w], pts[c][:, 0:w])
                        c0 += w

        # ---- phase 1b: V token-major (first nkt tiles), masked by z,
        # plus z as the ones-column for the softmax denominator ----
        with tc.tile_pool(name="ps_v", bufs=2, space="PSUM") as pv:
            for tt in range(nkt):
                pvt = pv.tile([PT, HC], F32, name="pvt", tag="pvt")
                for fc in range(NFC):
                    nc.tensor.matmul(
                        pvt[:],
                        xts[fc][:, tt * PT:(tt + 1) * PT],
                        wts[fc][:, 2 * HC:3 * HC],
                        start=(fc == 0), stop=(fc == NFC - 1),
                    )
                for h in range(HPC):
                    nc.vector.tensor_scalar_mul(
                        vzs[h][:, tt, 0:DK],
                        pvt[:, h * DK:(h + 1) * DK],
                        ztt[:, tt:tt + 1],
                    )
            for h in range(HPC):
                nc.vector.tensor_copy(vzs[h][:, :, DK], ztt[:, 0:nkt])

        # ---- phase 2: scores -> exp -> E @ Vz, q in halves of 1024 ----
        # Both heads of a pair are interleaved: their score matmuls have a
        # 64-row contraction at base partitions 0 / 64, so the PE runs them
        # concurrently (row tiling), and the exp of one head overlaps the
        # other head's matmuls, keeping the PE HAM-warm.
        # PSUM: st0+st1 [128,1024]f32 (2+2 banks) + pp0+pp1 [65,1024]f32
        # (2+2 banks) = 8. pp rows 0:64 = numerator, row 64 = denominator;
        # pp is evicted to SBUF with one copy so the divide chain
        # (recip -> broadcast -> mul) stays off the PV critical path.
        QH = 1024
        with tc.tile_pool(name="ps_s", bufs=1, space="PSUM") as pss, \
             tc.tile_pool(name="ps_p", bufs=1, space="PSUM") as psp, \
             tc.tile_pool(name="etp", bufs=nkt) as etp, \
             tc.tile_pool(name="upp", bufs=2) as upp, \
             tc.tile_pool(name="rpp", bufs=2) as rpp, \
             tc.tile_pool(name="bpp", bufs=2) as bpp, \
             tc.tile_pool(name="stg", bufs=3) as stg:

            def emit_a(g):
                # stage A: scores + exp for every kt; E tiles parked in SBUF.
                # Tile m covers q-span m*512; within a tile, head j owns
                # columns j*512:(j+1)*512 (its own PSUM bank).  The two
                # heads' score matmuls (64-row contraction at base
                # partitions 0 / 64) are emitted back-to-back so the PE
                # runs them concurrently via row tiling, and one ACTIVATE
                # covers both heads.
                qh, p = g
                q0 = qh * QH
                qa = [qsb[p][j * DK:(j + 1) * DK, :] for j in range(2)]
                ka = [ksb[p][j * DK:(j + 1) * DK, :] for j in range(2)]
                ets = []
                for kt in range(nkt):
                    pair = []
                    for m in range(2):
                        stm = pss.tile([PT, QH], F32, name=f"st{m}", tag=f"st{m}")
                        for j in range(2):
                            nc.tensor.matmul(
                                stm[:, j * 512:(j + 1) * 512],
                                ka[j][:, kt * PT:(kt + 1) * PT],
                                qa[j][:, q0 + m * 512:q0 + (m + 1) * 512],
                                start=True, stop=True,
                            )
                        etm = etp.tile([PT, QH], BF16, name=f"et{m}", tag=f"et{m}")
                        nc.scalar.activation(etm[:], stm[:], EXP)
                        pair.append(etm)
                    ets.append(pair)
                return ets

            def emit_b(g, ets):
                # stage B: PV accumulation + normalization; only pp banks,
                # so it overlaps the next group's stage A on the PE
                qh, p = g
                q0 = qh * QH
                pps = [
                    psp.tile([DK + 1, QH], F32, name=f"pp{j}", tag=f"pp{j}")
                    for j in range(2)
                ]
                for kt in range(nkt):
                    for m in range(2):
                        for j in range(2):
                            nc.tensor.matmul(
                                pps[j][:, m * 512:(m + 1) * 512],
                                vzs[2 * p + j][:, kt, :],
                                ets[kt][m][:, j * 512:(j + 1) * 512],
                                start=(kt == 0), stop=(kt == nkt - 1),
                            )
                for j in range(2):
                    po = j * DK
                    u = upp.tile([DK + 1, QH], F32, name=f"u{j}", tag=f"u{j}")
                    nc.vector.tensor_copy(u[:], pps[j][:])
                    # 1/den: DVE reciprocal is 8 cyc/elem per partition
                    # lane, so a [1, QH] recip is 8.5us of one lane. DMA
                    # the row into a [128, QH/128] layout, recip there
                    # (128 lanes -> ~70ns), DMA back, then broadcast.
                    # (ScalarE ln/exp would thrash the activation table
                    # set against the score exps; custom-DVE approx
                    # reciprocal is broken on this runtime.)
                    dvert = rpp.tile([PT, QH // PT], F32, name="dv", tag=f"dv{j}")
                    nc.sync.dma_start(dvert[:], u[DK:DK + 1, :])
                    nc.vector.reciprocal(dvert[:], dvert[:])
                    rr = rpp.tile([1, QH], F32, name="rr", tag=f"rr{j}")
                    nc.sync.dma_start(rr[:], dvert[:])
                    rb = bpp.tile([DK, QH], F32, name="rb", tag=f"rb{j}")
                    nc.gpsimd.partition_broadcast(rb[:], rr[:])
                    nc.vector.tensor_mul(
                        attnT[p][po:po + DK, q0:q0 + QH], u[0:DK, :], rb[:]
                    )

            def emit_out(tts):
                # phase 4: partial = attn @ W_o[:, rows].T for token blocks
                # whose attnT columns are complete.  pot reuses the st PSUM
                # tags (same 2-bank shape), so the qh=0 half overlaps the
                # trailing B stages on the PE.
                for tt in tts:
                    pot = pss.tile([PT, D], F32, name="pot", tag=f"st{tt % 2}")
                    for c in range(2):
                        for half in range(2):
                            nc.tensor.matmul(
                                pot[:, half * 512:(half + 1) * 512],
                                attnT[c][:, tt * PT:(tt + 1) * PT],
                                wos[c][:, half * 512:(half + 1) * 512],
                                start=(c == 0), stop=(c == 1),
                            )
                    so = stg.tile([PT, D], BF16, name="so", tag="so")
                    nc.vector.tensor_copy(so[:], pot[:])
                    nc.sync.dma_start(out[tt * PT:(tt + 1) * PT, :], so[:])

            e00 = emit_a((0, 0))
            e01 = emit_a((0, 1))
            emit_b((0, 0), e00)
            e10 = emit_a((1, 0))
            emit_b((0, 1), e01)
            e11 = emit_a((1, 1))
            emit_out(range(0, NTT // 2))
            emit_b((1, 0), e10)
            emit_b((1, 1), e11)
            emit_out(range(NTT // 2, NTT))


def build(nkt=NTT):
    nc = bacc.Bacc(
        "TRN2",
        target_bir_lowering=False,
        debug=False,
        enable_asserts=True,
        num_devices=N_CORES,
    )
    xt = nc.dram_tensor("xt", [D, S], BF16, kind="ExternalInput").ap()
    wqkv = nc.dram_tensor("wqkv", [D, 3 * HC], BF16, kind="ExternalInput").ap()
    wo = nc.dram_tensor("wo", [HC, D], BF16, kind="ExternalInput").ap()
    zt = nc.dram_tensor("zt", [PT, NTT], F32, kind="ExternalInput").ap()
    out = nc.dram_tensor("out", [S, D], BF16, kind="ExternalOutput").ap()
    with tile.TileContext(nc) as tc:
        _emit(tc, xt, wqkv, wo, zt, out, nkt)
    nc.compile()
    return nc


_NCS = {}


def _get_nc(nkt):
    if nkt not in _NCS:
        _NCS[nkt] = build(nkt)
    return _NCS[nkt]


def _nkt_for(vls):
    nkts = []
    for v in vls:
        v = int(v)
        nkts.append(NTT if v <= 0 else min(NTT, (v + PT - 1) // PT))
    return max(nkts)


def make_in_maps(X, valid_lens, W_q, W_k, W_v, W_o):
    import ml_dtypes

    bf16 = ml_dtypes.bfloat16
    X = np.asarray(X, dtype=np.float32)
    W_q = np.asarray(W_q, dtype=np.float32)
    W_k = np.asarray(W_k, dtype=np.float32)
    W_v = np.asarray(W_v, dtype=np.float32)
    W_o = np.asarray(W_o, dtype=np.float32)
    vls = np.asarray(valid_lens).astype(np.int64)
    in_maps = []
    for c in range(N_CORES):
        b, g = divmod(c, 4)
        rows = slice(g * HC, (g + 1) * HC)
        vl = int(vls[b])
        s = 0.125 if vl > 0 else 0.0
        if vl > 0:
            z = (np.arange(S) < vl).astype(np.float32)
        else:
            z = np.ones(S, dtype=np.float32)
        wqkv = np.concatenate(
            [W_q[rows].T * s, W_k[rows].T, W_v[rows].T], axis=1
        )
        in_maps.append({
            "xt": np.ascontiguousarray(X[b].T).astype(bf16),
            "wqkv": np.ascontiguousarray(wqkv).astype(bf16),
            "wo": np.ascontiguousarray(W_o.T[rows]).astype(bf16),
            "zt": np.ascontiguousarray(z.reshape(NTT, PT).T),
        })
    return in_maps


def combine(outs):
    out = np.empty((B, S, D), dtype=np.float32)
    for b in range(B):
        out[b] = (
            np.asarray(outs[4 * b], np.float32)
            + np.asarray(outs[4 * b + 1], np.float32)
            + np.asarray(outs[4 * b + 2], np.float32)
            + np.asarray(outs[4 * b + 3], np.float32)
        )
    return out


def kernel(X, valid_lens, W_q, W_k, W_v, W_o):
    vls = np.asarray(valid_lens).astype(np.int64)
    nc = _get_nc(_nkt_for(vls))
    in_maps = make_in_maps(X, valid_lens, W_q, W_k, W_v, W_o)
    res = bass_utils.run_bass_kernel_spmd(nc, in_maps, core_ids=list(range(N_CORES)))
    return combine([r["out"] for r in res.results])
